# revision 15
# baseline (speedup 1.0000x reference)
"""MoE kernel for trn2: expert-parallel over 8 NeuronCores.

Reference model (B,S,D,H,O,E,K = 4,1024,1024,2048,1024,8,2):
  gating: raw = x@Wg+bg; top-2 softmax -> combine weights
  experts: y_e = relu(relu(x@W1e+b1e)@W2e+b2e)@W3e+b3e
  out[t] = sum_e combine[t,e] * y_e[t]
  + aux outputs (load-balance loss, avg expert counts, gate entropy)
  (all derived from gating only)

Strategy: gating + dispatch on host (0.4% of FLOPs), one expert per core.
Each core computes its expert's MLP over only the tokens routed to it
(~T*K/E = 1024 of 4096 tokens -> 4x sparser than the reference's dense
grouped GEMM). Device matmuls in bf16 with fp32 PSUM accumulation.
"""

import os
import sys
import types

sys.path.insert(0, "/opt/trn_rl_repo")

import numpy as np
import ml_dtypes
from contextlib import ExitStack


def _ensure_ntff_hook():
    """Make trace=True work under axon if the image lacks antenv.axon_hooks."""
    try:
        from antenv.axon_hooks import get_axon_ntff_profile_hook  # noqa: F401
        return
    except ImportError:
        pass
    mod = types.ModuleType("antenv.axon_hooks")
    _hook = [None]
    mod.set_axon_ntff_profile_hook = lambda h: _hook.__setitem__(0, h)
    mod.get_axon_ntff_profile_hook = lambda: _hook[0]
    try:
        import antenv
        sys.modules["antenv.axon_hooks"] = mod
        antenv.axon_hooks = mod
        from trn_agent_boot.trn_boot import _ntff_profile_via_ctypes
        hook = _ntff_profile_via_ctypes("/opt/axon/libaxon_pjrt.so")
        if hook is not None:
            mod.set_axon_ntff_profile_hook(hook)
    except Exception:
        pass


_ensure_ntff_hook()

import concourse.bass as bass
import concourse.tile as tile
from concourse import bacc, mybir
from concourse.bass_utils import run_bass_kernel_spmd

# Problem dims (hardcoded per the harness contract)
B, S, D, H, O, E, K = 4, 1024, 1024, 2048, 1024, 8, 2
T = B * S
LOAD_BALANCE_WEIGHT = 0.01
P = 128
KD = D // P    # 8  k-chunks for layer 1
KH = H // P    # 16 k-chunks for layers 2/3
MH = H // P    # 16 m-blocks for layers 1/2
MO = O // P    # 8  m-blocks for layer 3
NCHUNK = 512   # PSUM bank = 512 fp32

F16 = mybir.dt.float16
F32 = mybir.dt.float32

_nc_cache: dict = {}
LAST_RESULT = None


def _chunks(c):
    return [(o, min(NCHUNK, c - o)) for o in range(0, c, NCHUNK)]


def build_nc(C: int):
    """One expert's MLP over C (padded) tokens: yT = MLP(xT) in m-blocks."""
    nc = bacc.Bacc("TRN2", target_bir_lowering=False, debug=False, num_devices=E)

    xp = nc.declare_dram_parameter("xp", [P, KD, C], F16, isOutput=False)
    w1p = nc.declare_dram_parameter("w1p", [MH, P, KD, P], F16, isOutput=False)
    w2p = nc.declare_dram_parameter("w2p", [MH, P, KH, P], F16, isOutput=False)
    w3p = nc.declare_dram_parameter("w3p", [MO, P, KH, P], F16, isOutput=False)
    b1p = nc.declare_dram_parameter("b1p", [P, MH], F32, isOutput=False)
    b2p = nc.declare_dram_parameter("b2p", [P, MH], F32, isOutput=False)
    b3p = nc.declare_dram_parameter("b3p", [P, MO], F32, isOutput=False)
    ytp = nc.declare_dram_parameter("ytp", [MO, P, C], F32, isOutput=True)

    nch = _chunks(C)

    with tile.TileContext(nc) as tc, ExitStack() as ctx:
        consts = ctx.enter_context(tc.tile_pool(name="consts", bufs=1))
        acts = ctx.enter_context(tc.tile_pool(name="acts", bufs=1))
        wpool = ctx.enter_context(tc.tile_pool(name="wpool", bufs=4))
        ypool = ctx.enter_context(tc.tile_pool(name="ypool", bufs=4))
        pspool = ctx.enter_context(tc.tile_pool(name="pspool", bufs=6, space="PSUM"))

        x_sb = acts.tile([P, KD, C], F16, tag="x")
        # split the load per k-chunk so layer 1 can start after the first
        # slice; sync engine = HWDGE (fast first-byte), parallel to weight
        # loads on scalar's HWDGE ring
        for k in range(KD):
            nc.sync.dma_start(out=x_sb[:, k, :], in_=xp[:, k, :])

        b1_sb = consts.tile([P, MH], F32, tag="b1")
        b2_sb = consts.tile([P, MH], F32, tag="b2")
        b3_sb = consts.tile([P, MO], F32, tag="b3")
        nc.gpsimd.dma_start(out=b1_sb, in_=b1p[:, :])
        nc.gpsimd.dma_start(out=b2_sb, in_=b2p[:, :])
        nc.gpsimd.dma_start(out=b3_sb, in_=b3p[:, :])

        h1_sb = acts.tile([P, MH, C], F16, tag="h1")
        h2_sb = acts.tile([P, MH, C], F16, tag="h2")

        def layer(w_param, nk, nm, rhs_sb, bias_sb, wtag, evict):
            for m in range(nm):
                w_sb = wpool.tile([P, nk, P], F16, tag=wtag)
                nc.scalar.dma_start(out=w_sb, in_=w_param[m])
                psums = []
                for n, (off, nsz) in enumerate(nch):
                    ps_t = pspool.tile([P, NCHUNK], F32, tag="ps", name=f"ps_{m}_{n}")
                    psums.append(ps_t)
                for k in range(nk):
                    for n, (off, nsz) in enumerate(nch):
                        nc.tensor.matmul(
                            psums[n][:, :nsz],
                            w_sb[:, k, :],
                            rhs_sb[:, k, off:off + nsz],
                            start=(k == 0),
                            stop=(k == nk - 1),
                        )
                for n, (off, nsz) in enumerate(nch):
                    evict(m, off, nsz, psums[n])

        def layer1_nouter(w_param, rhs_sb, evict):
            # n-outer: the first matmul only needs x k-slice 0, and each
            # x k-slice is consumed over a full 16-m sweep (~3.4us), so x
            # DMA never stalls the PE. w1 stays resident (loaded once).
            # sync queue: shared with x (2.2MB, done ~12us) but NOT with the
            # eager w2 prefetches on scalar's queue, which would starve the
            # early w1 m-blocks the L1 m-sweep needs just-in-time
            w_sb = acts.tile([P, KD, H], F16, tag="w1r", name="w1_resident")
            for m in range(MH):
                nc.gpsimd.dma_start(out=w_sb[:, :, m * P:(m + 1) * P],
                                    in_=w_param[m])
            for n, (off, nsz) in enumerate(nch):
                for m in range(MH):
                    ps_t = pspool.tile([P, NCHUNK], F32, tag="ps",
                                       name=f"l1ps_{m}_{n}")
                    for k in range(KD):
                        nc.tensor.matmul(
                            ps_t[:, :nsz],
                            w_sb[:, k, m * P:(m + 1) * P],
                            rhs_sb[:, k, off:off + nsz],
                            start=(k == 0),
                            stop=(k == KD - 1),
                        )
                    evict(m, off, nsz, ps_t)

        def evict_relu(h_sb, bias_sb):
            def f(m, off, nsz, ps):
                nc.scalar.activation(
                    h_sb[:, m, off:off + nsz], ps[:, :nsz],
                    mybir.ActivationFunctionType.Relu,
                    bias=bias_sb[:, m:m + 1],
                )
            return f

        def evict_out(m, off, nsz, ps):
            y_sb = ypool.tile([P, NCHUNK], F32, tag="y")
            nc.scalar.activation(
                y_sb[:, :nsz], ps[:, :nsz],
                mybir.ActivationFunctionType.Identity,
                bias=b3_sb[:, m:m + 1],
            )
            nc.gpsimd.dma_start(out=ytp[m][:, off:off + nsz], in_=y_sb[:, :nsz])

        layer1_nouter(w1p, x_sb, evict_relu(h1_sb, b1_sb))
        layer(w2p, KH, MH, h1_sb, b2_sb, "w2", evict_relu(h2_sb, b2_sb))
        layer(w3p, KH, MO, h2_sb, b3_sb, "w3", evict_out)

    nc.compile()
    return nc


def _get_nc(C: int):
    if C not in _nc_cache:
        _nc_cache[C] = build_nc(C)
    return _nc_cache[C]


def _pack_w(w, nk, nm):
    """[nk*P, nm*P] fp32 -> [nm, P, nk, P] bf16 (m-block-major, partition-major)."""
    wb = w.astype(np.float16)
    return np.ascontiguousarray(
        wb.reshape(nk, P, nm, P).transpose(2, 1, 0, 3)
    )


def _pack_b(b, nm):
    """[nm*P] fp32 -> [P, nm] fp32."""
    return np.ascontiguousarray(b.astype(np.float32).reshape(nm, P).T)


def _softmax(x, axis=-1):
    m = np.max(x, axis=axis, keepdims=True)
    e = np.exp(x - m)
    return e / np.sum(e, axis=axis, keepdims=True)


def kernel(x, Wg, bg, W1, b1, W2, b2, W3, b3):
    x = np.asarray(x, dtype=np.float32)
    Wg = np.asarray(Wg, dtype=np.float32)
    bg = np.asarray(bg, dtype=np.float32)
    W1 = np.asarray(W1, dtype=np.float32)
    b1 = np.asarray(b1, dtype=np.float32)
    W2 = np.asarray(W2, dtype=np.float32)
    b2 = np.asarray(b2, dtype=np.float32)
    W3 = np.asarray(W3, dtype=np.float32)
    b3 = np.asarray(b3, dtype=np.float32)

    xt = x.reshape(T, D)

    # ---- gating (host, exact fp32) ----
    raw_gates = xt @ Wg + bg                                   # [T, E]
    topk_idx = np.argpartition(raw_gates, E - K, axis=-1)[:, E - K:]   # [T, K]
    topk_val = np.take_along_axis(raw_gates, topk_idx, axis=-1)
    gates = _softmax(topk_val, axis=-1)                        # [T, K]

    combine = np.zeros((T, E), dtype=np.float32)
    np.put_along_axis(combine, topk_idx, gates, axis=-1)

    # ---- aux outputs (gating-only) ----
    gate_probs = _softmax(raw_gates, axis=-1)                  # [T, E]
    avg_gate_probs = gate_probs.mean(axis=0, dtype=np.float32)
    avg_expert_counts = combine.mean(axis=0, dtype=np.float32)
    load_balance_loss = np.float32(
        LOAD_BALANCE_WEIGHT * E * np.sum(avg_gate_probs * avg_expert_counts,
                                         dtype=np.float32))
    gate_entropy = np.float32(
        -np.sum(avg_gate_probs * np.log(avg_gate_probs + 1e-8),
                dtype=np.float32))

    # ---- dispatch: token lists per expert ----
    tok_idx = [np.nonzero(combine[:, e] > 0)[0] for e in range(E)]
    counts = [len(ti) for ti in tok_idx]
    # capacity = exact max count rounded to 16 (remainder matmul chunks run
    # at full rate, so padding cost is linear in C)
    C = max(128, ((max(counts) + 15) // 16) * 16)

    xb = xt.astype(np.float16)
    in_maps = []
    for e in range(E):
        ti = tok_idx[e]
        xe = np.zeros((P, KD, C), dtype=np.float16)
        # gathered tokens, feature-major: [D, cnt] -> [P, KD, cnt]
        xe_g = xb[ti].T.reshape(KD, P, len(ti)).transpose(1, 0, 2)
        xe[:, :, :len(ti)] = xe_g
        in_maps.append({
            "xp": xe,
            "w1p": _pack_w(W1[e], KD, MH),
            "w2p": _pack_w(W2[e], KH, MH),
            "w3p": _pack_w(W3[e], KH, MO),
            "b1p": _pack_b(b1[e], MH),
            "b2p": _pack_b(b2[e], MH),
            "b3p": _pack_b(b3[e], MO),
        })

    # ---- run expert MLPs on the 8 cores ----
    nc = _get_nc(C)
    res = run_bass_kernel_spmd(nc, in_maps, list(range(E)))
    global LAST_RESULT
    LAST_RESULT = res

    # ---- combine with gates (host) ----
    out = np.zeros((T, O), dtype=np.float32)
    for e in range(E):
        ti = tok_idx[e]
        yt = res.results[e]["ytp"].reshape(O, C)   # [O, C]
        w_e = combine[ti, e]
        out[ti] += w_e[:, None] * yt[:, :len(ti)].T

    return (out.reshape(B, S, O), load_balance_loss, avg_expert_counts,
            gate_entropy)


# revision 16
# speedup vs baseline: 1.0304x; 1.0304x over previous
"""MoE kernel for trn2: expert-parallel over 8 NeuronCores.

Reference model (B,S,D,H,O,E,K = 4,1024,1024,2048,1024,8,2):
  gating: raw = x@Wg+bg; top-2 softmax -> combine weights
  experts: y_e = relu(relu(x@W1e+b1e)@W2e+b2e)@W3e+b3e
  out[t] = sum_e combine[t,e] * y_e[t]
  + aux outputs (load-balance loss, avg expert counts, gate entropy)
  (all derived from gating only)

Strategy: gating + dispatch on host (0.4% of FLOPs), one expert per core.
Each core computes its expert's MLP over only the tokens routed to it
(~T*K/E = 1024 of 4096 tokens -> 4x sparser than the reference's dense
grouped GEMM). Device matmuls in bf16 with fp32 PSUM accumulation.
"""

import os
import sys
import types

sys.path.insert(0, "/opt/trn_rl_repo")

import numpy as np
import ml_dtypes
from contextlib import ExitStack


def _ensure_ntff_hook():
    """Make trace=True work under axon if the image lacks antenv.axon_hooks."""
    try:
        from antenv.axon_hooks import get_axon_ntff_profile_hook  # noqa: F401
        return
    except ImportError:
        pass
    mod = types.ModuleType("antenv.axon_hooks")
    _hook = [None]
    mod.set_axon_ntff_profile_hook = lambda h: _hook.__setitem__(0, h)
    mod.get_axon_ntff_profile_hook = lambda: _hook[0]
    try:
        import antenv
        sys.modules["antenv.axon_hooks"] = mod
        antenv.axon_hooks = mod
        from trn_agent_boot.trn_boot import _ntff_profile_via_ctypes
        hook = _ntff_profile_via_ctypes("/opt/axon/libaxon_pjrt.so")
        if hook is not None:
            mod.set_axon_ntff_profile_hook(hook)
    except Exception:
        pass


_ensure_ntff_hook()

import concourse.bass as bass
import concourse.tile as tile
from concourse import bacc, mybir
from concourse.bass_utils import run_bass_kernel_spmd

# Problem dims (hardcoded per the harness contract)
B, S, D, H, O, E, K = 4, 1024, 1024, 2048, 1024, 8, 2
T = B * S
LOAD_BALANCE_WEIGHT = 0.01
P = 128
KD = D // P    # 8  k-chunks for layer 1
KH = H // P    # 16 k-chunks for layers 2/3
MH = H // P    # 16 m-blocks for layers 1/2
MO = O // P    # 8  m-blocks for layer 3
NCHUNK = 512   # PSUM bank = 512 fp32

F16 = mybir.dt.float16
F32 = mybir.dt.float32

_nc_cache: dict = {}
LAST_RESULT = None


def _chunks(c):
    return [(o, min(NCHUNK, c - o)) for o in range(0, c, NCHUNK)]


def build_nc(C: int):
    """One expert's MLP over C (padded) tokens: yT = MLP(xT) in m-blocks."""
    nc = bacc.Bacc("TRN2", target_bir_lowering=False, debug=False, num_devices=E)

    xp = nc.declare_dram_parameter("xp", [P, KD, C], F16, isOutput=False)
    w1p = nc.declare_dram_parameter("w1p", [MH, P, KD, P], F16, isOutput=False)
    w2p = nc.declare_dram_parameter("w2p", [MH, P, KH, P], F16, isOutput=False)
    w3p = nc.declare_dram_parameter("w3p", [MO, P, KH, P], F16, isOutput=False)
    b1p = nc.declare_dram_parameter("b1p", [P, MH], F32, isOutput=False)
    b2p = nc.declare_dram_parameter("b2p", [P, MH], F32, isOutput=False)
    b3p = nc.declare_dram_parameter("b3p", [P, MO], F32, isOutput=False)
    ytp = nc.declare_dram_parameter("ytp", [MO, P, C], F32, isOutput=True)

    nch = _chunks(C)

    with tile.TileContext(nc) as tc, ExitStack() as ctx:
        consts = ctx.enter_context(tc.tile_pool(name="consts", bufs=1))
        acts = ctx.enter_context(tc.tile_pool(name="acts", bufs=1))
        wpool = ctx.enter_context(tc.tile_pool(name="wpool", bufs=4))
        ypool = ctx.enter_context(tc.tile_pool(name="ypool", bufs=4))
        pspool = ctx.enter_context(tc.tile_pool(name="pspool", bufs=6, space="PSUM"))

        x_sb = acts.tile([P, KD, C], F16, tag="x")
        # split the load per k-chunk so layer 1 can start after the first
        # slice; sync engine = HWDGE (fast first-byte), parallel to weight
        # loads on scalar's HWDGE ring
        for k in range(KD):
            nc.sync.dma_start(out=x_sb[:, k, :], in_=xp[:, k, :])

        b1_sb = consts.tile([P, MH], F32, tag="b1")
        b2_sb = consts.tile([P, MH], F32, tag="b2")
        b3_sb = consts.tile([P, MO], F32, tag="b3")
        nc.gpsimd.dma_start(out=b1_sb, in_=b1p[:, :])
        nc.gpsimd.dma_start(out=b2_sb, in_=b2p[:, :])
        nc.gpsimd.dma_start(out=b3_sb, in_=b3p[:, :])

        h1_sb = acts.tile([P, MH, C], F16, tag="h1")
        h2_sb = acts.tile([P, MH, C], F16, tag="h2")

        def layer(w_param, nk, nm, rhs_sb, bias_sb, wtag, evict):
            for m in range(nm):
                w_sb = wpool.tile([P, nk, P], F16, tag=wtag)
                nc.sync.dma_start(out=w_sb, in_=w_param[m])
                psums = []
                for n, (off, nsz) in enumerate(nch):
                    ps_t = pspool.tile([P, NCHUNK], F32, tag="ps", name=f"ps_{m}_{n}")
                    psums.append(ps_t)
                for k in range(nk):
                    for n, (off, nsz) in enumerate(nch):
                        nc.tensor.matmul(
                            psums[n][:, :nsz],
                            w_sb[:, k, :],
                            rhs_sb[:, k, off:off + nsz],
                            start=(k == 0),
                            stop=(k == nk - 1),
                        )
                for n, (off, nsz) in enumerate(nch):
                    evict(m, off, nsz, psums[n])

        def layer1_nouter(w_param, rhs_sb, evict):
            # n-outer: the first matmul only needs x k-slice 0, and each
            # x k-slice is consumed over a full 16-m sweep (~3.4us), so x
            # DMA never stalls the PE. w1 stays resident (loaded once).
            # sync queue: shared with x (2.2MB, done ~12us) but NOT with the
            # eager w2 prefetches on scalar's queue, which would starve the
            # early w1 m-blocks the L1 m-sweep needs just-in-time
            w_sb = acts.tile([P, KD, H], F16, tag="w1r", name="w1_resident")
            for m in range(MH):
                nc.scalar.dma_start(out=w_sb[:, :, m * P:(m + 1) * P],
                                    in_=w_param[m])
            for n, (off, nsz) in enumerate(nch):
                for m in range(MH):
                    ps_t = pspool.tile([P, NCHUNK], F32, tag="ps",
                                       name=f"l1ps_{m}_{n}")
                    for k in range(KD):
                        nc.tensor.matmul(
                            ps_t[:, :nsz],
                            w_sb[:, k, m * P:(m + 1) * P],
                            rhs_sb[:, k, off:off + nsz],
                            start=(k == 0),
                            stop=(k == KD - 1),
                        )
                    evict(m, off, nsz, ps_t)

        def evict_relu(h_sb, bias_sb):
            def f(m, off, nsz, ps):
                nc.scalar.activation(
                    h_sb[:, m, off:off + nsz], ps[:, :nsz],
                    mybir.ActivationFunctionType.Relu,
                    bias=bias_sb[:, m:m + 1],
                )
            return f

        def evict_out(m, off, nsz, ps):
            y_sb = ypool.tile([P, NCHUNK], F32, tag="y")
            nc.scalar.activation(
                y_sb[:, :nsz], ps[:, :nsz],
                mybir.ActivationFunctionType.Identity,
                bias=b3_sb[:, m:m + 1],
            )
            nc.gpsimd.dma_start(out=ytp[m][:, off:off + nsz], in_=y_sb[:, :nsz])

        layer1_nouter(w1p, x_sb, evict_relu(h1_sb, b1_sb))
        layer(w2p, KH, MH, h1_sb, b2_sb, "w2", evict_relu(h2_sb, b2_sb))
        layer(w3p, KH, MO, h2_sb, b3_sb, "w3", evict_out)

    nc.compile()
    return nc


def _get_nc(C: int):
    if C not in _nc_cache:
        _nc_cache[C] = build_nc(C)
    return _nc_cache[C]


def _pack_w(w, nk, nm):
    """[nk*P, nm*P] fp32 -> [nm, P, nk, P] bf16 (m-block-major, partition-major)."""
    wb = w.astype(np.float16)
    return np.ascontiguousarray(
        wb.reshape(nk, P, nm, P).transpose(2, 1, 0, 3)
    )


def _pack_b(b, nm):
    """[nm*P] fp32 -> [P, nm] fp32."""
    return np.ascontiguousarray(b.astype(np.float32).reshape(nm, P).T)


def _softmax(x, axis=-1):
    m = np.max(x, axis=axis, keepdims=True)
    e = np.exp(x - m)
    return e / np.sum(e, axis=axis, keepdims=True)


def kernel(x, Wg, bg, W1, b1, W2, b2, W3, b3):
    x = np.asarray(x, dtype=np.float32)
    Wg = np.asarray(Wg, dtype=np.float32)
    bg = np.asarray(bg, dtype=np.float32)
    W1 = np.asarray(W1, dtype=np.float32)
    b1 = np.asarray(b1, dtype=np.float32)
    W2 = np.asarray(W2, dtype=np.float32)
    b2 = np.asarray(b2, dtype=np.float32)
    W3 = np.asarray(W3, dtype=np.float32)
    b3 = np.asarray(b3, dtype=np.float32)

    xt = x.reshape(T, D)

    # ---- gating (host, exact fp32) ----
    raw_gates = xt @ Wg + bg                                   # [T, E]
    topk_idx = np.argpartition(raw_gates, E - K, axis=-1)[:, E - K:]   # [T, K]
    topk_val = np.take_along_axis(raw_gates, topk_idx, axis=-1)
    gates = _softmax(topk_val, axis=-1)                        # [T, K]

    combine = np.zeros((T, E), dtype=np.float32)
    np.put_along_axis(combine, topk_idx, gates, axis=-1)

    # ---- aux outputs (gating-only) ----
    gate_probs = _softmax(raw_gates, axis=-1)                  # [T, E]
    avg_gate_probs = gate_probs.mean(axis=0, dtype=np.float32)
    avg_expert_counts = combine.mean(axis=0, dtype=np.float32)
    load_balance_loss = np.float32(
        LOAD_BALANCE_WEIGHT * E * np.sum(avg_gate_probs * avg_expert_counts,
                                         dtype=np.float32))
    gate_entropy = np.float32(
        -np.sum(avg_gate_probs * np.log(avg_gate_probs + 1e-8),
                dtype=np.float32))

    # ---- dispatch: token lists per expert ----
    tok_idx = [np.nonzero(combine[:, e] > 0)[0] for e in range(E)]
    counts = [len(ti) for ti in tok_idx]
    # capacity = exact max count rounded to 16 (remainder matmul chunks run
    # at full rate, so padding cost is linear in C)
    C = max(128, ((max(counts) + 15) // 16) * 16)

    xb = xt.astype(np.float16)
    in_maps = []
    for e in range(E):
        ti = tok_idx[e]
        xe = np.zeros((P, KD, C), dtype=np.float16)
        # gathered tokens, feature-major: [D, cnt] -> [P, KD, cnt]
        xe_g = xb[ti].T.reshape(KD, P, len(ti)).transpose(1, 0, 2)
        xe[:, :, :len(ti)] = xe_g
        in_maps.append({
            "xp": xe,
            "w1p": _pack_w(W1[e], KD, MH),
            "w2p": _pack_w(W2[e], KH, MH),
            "w3p": _pack_w(W3[e], KH, MO),
            "b1p": _pack_b(b1[e], MH),
            "b2p": _pack_b(b2[e], MH),
            "b3p": _pack_b(b3[e], MO),
        })

    # ---- run expert MLPs on the 8 cores ----
    nc = _get_nc(C)
    res = run_bass_kernel_spmd(nc, in_maps, list(range(E)))
    global LAST_RESULT
    LAST_RESULT = res

    # ---- combine with gates (host) ----
    out = np.zeros((T, O), dtype=np.float32)
    for e in range(E):
        ti = tok_idx[e]
        yt = res.results[e]["ytp"].reshape(O, C)   # [O, C]
        w_e = combine[ti, e]
        out[ti] += w_e[:, None] * yt[:, :len(ti)].T

    return (out.reshape(B, S, O), load_balance_loss, avg_expert_counts,
            gate_entropy)


# revision 17
# speedup vs baseline: 1.0714x; 1.0398x over previous
"""MoE kernel for trn2: expert-parallel over 8 NeuronCores.

Reference model (B,S,D,H,O,E,K = 4,1024,1024,2048,1024,8,2):
  gating: raw = x@Wg+bg; top-2 softmax -> combine weights
  experts: y_e = relu(relu(x@W1e+b1e)@W2e+b2e)@W3e+b3e
  out[t] = sum_e combine[t,e] * y_e[t]
  + aux outputs (load-balance loss, avg expert counts, gate entropy)
  (all derived from gating only)

Strategy: gating + dispatch on host (0.4% of FLOPs), one expert per core.
Each core computes its expert's MLP over only the tokens routed to it
(~T*K/E = 1024 of 4096 tokens -> 4x sparser than the reference's dense
grouped GEMM). Device matmuls in bf16 with fp32 PSUM accumulation.
"""

import os
import sys
import types

sys.path.insert(0, "/opt/trn_rl_repo")

import numpy as np
import ml_dtypes
from contextlib import ExitStack


def _ensure_ntff_hook():
    """Make trace=True work under axon if the image lacks antenv.axon_hooks."""
    try:
        from antenv.axon_hooks import get_axon_ntff_profile_hook  # noqa: F401
        return
    except ImportError:
        pass
    mod = types.ModuleType("antenv.axon_hooks")
    _hook = [None]
    mod.set_axon_ntff_profile_hook = lambda h: _hook.__setitem__(0, h)
    mod.get_axon_ntff_profile_hook = lambda: _hook[0]
    try:
        import antenv
        sys.modules["antenv.axon_hooks"] = mod
        antenv.axon_hooks = mod
        from trn_agent_boot.trn_boot import _ntff_profile_via_ctypes
        hook = _ntff_profile_via_ctypes("/opt/axon/libaxon_pjrt.so")
        if hook is not None:
            mod.set_axon_ntff_profile_hook(hook)
    except Exception:
        pass


_ensure_ntff_hook()

import concourse.bass as bass
import concourse.tile as tile
from concourse import bacc, mybir
from concourse.bass_utils import run_bass_kernel_spmd

# Problem dims (hardcoded per the harness contract)
B, S, D, H, O, E, K = 4, 1024, 1024, 2048, 1024, 8, 2
T = B * S
LOAD_BALANCE_WEIGHT = 0.01
P = 128
KD = D // P    # 8  k-chunks for layer 1
KH = H // P    # 16 k-chunks for layers 2/3
MH = H // P    # 16 m-blocks for layers 1/2
MO = O // P    # 8  m-blocks for layer 3
NCHUNK = 512   # PSUM bank = 512 fp32

F16 = mybir.dt.float16
F32 = mybir.dt.float32

_nc_cache: dict = {}
LAST_RESULT = None


def _chunks(c):
    return [(o, min(NCHUNK, c - o)) for o in range(0, c, NCHUNK)]


def build_nc(C: int):
    """One expert's MLP over C (padded) tokens: yT = MLP(xT) in m-blocks."""
    nc = bacc.Bacc("TRN2", target_bir_lowering=False, debug=False, num_devices=E)

    xp = nc.declare_dram_parameter("xp", [P, KD, C], F16, isOutput=False)
    w1p = nc.declare_dram_parameter("w1p", [MH, P, KD, P], F16, isOutput=False)
    w2p = nc.declare_dram_parameter("w2p", [MH, P, KH, P], F16, isOutput=False)
    w3p = nc.declare_dram_parameter("w3p", [MO, P, KH, P], F16, isOutput=False)
    b1p = nc.declare_dram_parameter("b1p", [P, MH], F32, isOutput=False)
    b2p = nc.declare_dram_parameter("b2p", [P, MH], F32, isOutput=False)
    b3p = nc.declare_dram_parameter("b3p", [P, MO], F32, isOutput=False)
    ytp = nc.declare_dram_parameter("ytp", [MO, P, C], F32, isOutput=True)

    nch = _chunks(C)

    with tile.TileContext(nc) as tc, ExitStack() as ctx:
        consts = ctx.enter_context(tc.tile_pool(name="consts", bufs=1))
        acts = ctx.enter_context(tc.tile_pool(name="acts", bufs=1))
        wpool = ctx.enter_context(tc.tile_pool(name="wpool", bufs=4))
        ypool = ctx.enter_context(tc.tile_pool(name="ypool", bufs=4))
        pspool = ctx.enter_context(tc.tile_pool(name="pspool", bufs=6, space="PSUM"))

        x_sb = acts.tile([P, KD, C], F16, tag="x")
        # split the load per k-chunk so layer 1 can start after the first
        # slice; sync engine = HWDGE (fast first-byte), parallel to weight
        # loads on scalar's HWDGE ring
        for k in range(KD):
            nc.sync.dma_start(out=x_sb[:, k, :], in_=xp[:, k, :])

        b1_sb = consts.tile([P, MH], F32, tag="b1")
        b2_sb = consts.tile([P, MH], F32, tag="b2")
        b3_sb = consts.tile([P, MO], F32, tag="b3")
        nc.gpsimd.dma_start(out=b1_sb, in_=b1p[:, :])
        nc.gpsimd.dma_start(out=b2_sb, in_=b2p[:, :])
        nc.gpsimd.dma_start(out=b3_sb, in_=b3p[:, :])

        h1_sb = acts.tile([P, MH, C], F16, tag="h1")
        h2_sb = acts.tile([P, MH, C], F16, tag="h2")

        def layer(w_param, nk, nm, rhs_sb, bias_sb, wtag, evict):
            for m in range(nm):
                w_sb = wpool.tile([P, nk, P], F16, tag=wtag)
                nc.scalar.dma_start(out=w_sb, in_=w_param[m])
                psums = []
                for n, (off, nsz) in enumerate(nch):
                    ps_t = pspool.tile([P, NCHUNK], F32, tag="ps", name=f"ps_{m}_{n}")
                    psums.append(ps_t)
                for k in range(nk):
                    for n, (off, nsz) in enumerate(nch):
                        nc.tensor.matmul(
                            psums[n][:, :nsz],
                            w_sb[:, k, :],
                            rhs_sb[:, k, off:off + nsz],
                            start=(k == 0),
                            stop=(k == nk - 1),
                        )
                for n, (off, nsz) in enumerate(nch):
                    evict(m, off, nsz, psums[n])

        def layer1_nouter(w_param, rhs_sb, evict):
            # n-outer: the first matmul only needs x k-slice 0, and each
            # x k-slice is consumed over a full 16-m sweep (~3.4us), so x
            # DMA never stalls the PE. w1 stays resident (loaded once).
            # sync queue: shared with x (2.2MB, done ~12us) but NOT with the
            # eager w2 prefetches on scalar's queue, which would starve the
            # early w1 m-blocks the L1 m-sweep needs just-in-time
            w_sb = acts.tile([P, KD, H], F16, tag="w1r", name="w1_resident")
            for m in range(MH):
                nc.scalar.dma_start(out=w_sb[:, :, m * P:(m + 1) * P],
                                    in_=w_param[m])
            for n, (off, nsz) in enumerate(nch):
                for m in range(MH):
                    ps_t = pspool.tile([P, NCHUNK], F32, tag="ps",
                                       name=f"l1ps_{m}_{n}")
                    for k in range(KD):
                        nc.tensor.matmul(
                            ps_t[:, :nsz],
                            w_sb[:, k, m * P:(m + 1) * P],
                            rhs_sb[:, k, off:off + nsz],
                            start=(k == 0),
                            stop=(k == KD - 1),
                        )
                    evict(m, off, nsz, ps_t)

        def evict_relu(h_sb, bias_sb):
            def f(m, off, nsz, ps):
                nc.scalar.activation(
                    h_sb[:, m, off:off + nsz], ps[:, :nsz],
                    mybir.ActivationFunctionType.Relu,
                    bias=bias_sb[:, m:m + 1],
                )
            return f

        def evict_out(m, off, nsz, ps):
            y_sb = ypool.tile([P, NCHUNK], F32, tag="y")
            nc.scalar.activation(
                y_sb[:, :nsz], ps[:, :nsz],
                mybir.ActivationFunctionType.Identity,
                bias=b3_sb[:, m:m + 1],
            )
            nc.gpsimd.dma_start(out=ytp[m][:, off:off + nsz], in_=y_sb[:, :nsz])

        layer1_nouter(w1p, x_sb, evict_relu(h1_sb, b1_sb))
        layer(w2p, KH, MH, h1_sb, b2_sb, "w2", evict_relu(h2_sb, b2_sb))
        layer(w3p, KH, MO, h2_sb, b3_sb, "w3", evict_out)

    nc.compile()
    return nc


def _get_nc(C: int):
    if C not in _nc_cache:
        _nc_cache[C] = build_nc(C)
    return _nc_cache[C]


def _pack_w(w, nk, nm):
    """[nk*P, nm*P] fp32 -> [nm, P, nk, P] bf16 (m-block-major, partition-major)."""
    wb = w.astype(np.float16)
    return np.ascontiguousarray(
        wb.reshape(nk, P, nm, P).transpose(2, 1, 0, 3)
    )


def _pack_b(b, nm):
    """[nm*P] fp32 -> [P, nm] fp32."""
    return np.ascontiguousarray(b.astype(np.float32).reshape(nm, P).T)


def _softmax(x, axis=-1):
    m = np.max(x, axis=axis, keepdims=True)
    e = np.exp(x - m)
    return e / np.sum(e, axis=axis, keepdims=True)


def kernel(x, Wg, bg, W1, b1, W2, b2, W3, b3):
    x = np.asarray(x, dtype=np.float32)
    Wg = np.asarray(Wg, dtype=np.float32)
    bg = np.asarray(bg, dtype=np.float32)
    W1 = np.asarray(W1, dtype=np.float32)
    b1 = np.asarray(b1, dtype=np.float32)
    W2 = np.asarray(W2, dtype=np.float32)
    b2 = np.asarray(b2, dtype=np.float32)
    W3 = np.asarray(W3, dtype=np.float32)
    b3 = np.asarray(b3, dtype=np.float32)

    xt = x.reshape(T, D)

    # ---- gating (host, exact fp32) ----
    raw_gates = xt @ Wg + bg                                   # [T, E]
    topk_idx = np.argpartition(raw_gates, E - K, axis=-1)[:, E - K:]   # [T, K]
    topk_val = np.take_along_axis(raw_gates, topk_idx, axis=-1)
    gates = _softmax(topk_val, axis=-1)                        # [T, K]

    combine = np.zeros((T, E), dtype=np.float32)
    np.put_along_axis(combine, topk_idx, gates, axis=-1)

    # ---- aux outputs (gating-only) ----
    gate_probs = _softmax(raw_gates, axis=-1)                  # [T, E]
    avg_gate_probs = gate_probs.mean(axis=0, dtype=np.float32)
    avg_expert_counts = combine.mean(axis=0, dtype=np.float32)
    load_balance_loss = np.float32(
        LOAD_BALANCE_WEIGHT * E * np.sum(avg_gate_probs * avg_expert_counts,
                                         dtype=np.float32))
    gate_entropy = np.float32(
        -np.sum(avg_gate_probs * np.log(avg_gate_probs + 1e-8),
                dtype=np.float32))

    # ---- dispatch: token lists per expert ----
    tok_idx = [np.nonzero(combine[:, e] > 0)[0] for e in range(E)]
    counts = [len(ti) for ti in tok_idx]
    # capacity = exact max count rounded to 16 (remainder matmul chunks run
    # at full rate, so padding cost is linear in C)
    C = max(128, ((max(counts) + 15) // 16) * 16)

    xb = xt.astype(np.float16)
    in_maps = []
    for e in range(E):
        ti = tok_idx[e]
        xe = np.zeros((P, KD, C), dtype=np.float16)
        # gathered tokens, feature-major: [D, cnt] -> [P, KD, cnt]
        xe_g = xb[ti].T.reshape(KD, P, len(ti)).transpose(1, 0, 2)
        xe[:, :, :len(ti)] = xe_g
        in_maps.append({
            "xp": xe,
            "w1p": _pack_w(W1[e], KD, MH),
            "w2p": _pack_w(W2[e], KH, MH),
            "w3p": _pack_w(W3[e], KH, MO),
            "b1p": _pack_b(b1[e], MH),
            "b2p": _pack_b(b2[e], MH),
            "b3p": _pack_b(b3[e], MO),
        })

    # ---- run expert MLPs on the 8 cores ----
    nc = _get_nc(C)
    res = run_bass_kernel_spmd(nc, in_maps, list(range(E)))
    global LAST_RESULT
    LAST_RESULT = res

    # ---- combine with gates (host) ----
    out = np.zeros((T, O), dtype=np.float32)
    for e in range(E):
        ti = tok_idx[e]
        yt = res.results[e]["ytp"].reshape(O, C)   # [O, C]
        w_e = combine[ti, e]
        out[ti] += w_e[:, None] * yt[:, :len(ti)].T

    return (out.reshape(B, S, O), load_balance_loss, avg_expert_counts,
            gate_entropy)


# revision 18
# speedup vs baseline: 1.0847x; 1.0124x over previous
"""MoE kernel for trn2: expert-parallel over 8 NeuronCores.

Reference model (B,S,D,H,O,E,K = 4,1024,1024,2048,1024,8,2):
  gating: raw = x@Wg+bg; top-2 softmax -> combine weights
  experts: y_e = relu(relu(x@W1e+b1e)@W2e+b2e)@W3e+b3e
  out[t] = sum_e combine[t,e] * y_e[t]
  + aux outputs (load-balance loss, avg expert counts, gate entropy)
  (all derived from gating only)

Strategy: gating + dispatch on host (0.4% of FLOPs), one expert per core.
Each core computes its expert's MLP over only the tokens routed to it
(~T*K/E = 1024 of 4096 tokens -> 4x sparser than the reference's dense
grouped GEMM). Device matmuls in bf16 with fp32 PSUM accumulation.
"""

import os
import sys
import types

sys.path.insert(0, "/opt/trn_rl_repo")

import numpy as np
import ml_dtypes
from contextlib import ExitStack


def _ensure_ntff_hook():
    """Make trace=True work under axon if the image lacks antenv.axon_hooks."""
    try:
        from antenv.axon_hooks import get_axon_ntff_profile_hook  # noqa: F401
        return
    except ImportError:
        pass
    mod = types.ModuleType("antenv.axon_hooks")
    _hook = [None]
    mod.set_axon_ntff_profile_hook = lambda h: _hook.__setitem__(0, h)
    mod.get_axon_ntff_profile_hook = lambda: _hook[0]
    try:
        import antenv
        sys.modules["antenv.axon_hooks"] = mod
        antenv.axon_hooks = mod
        from trn_agent_boot.trn_boot import _ntff_profile_via_ctypes
        hook = _ntff_profile_via_ctypes("/opt/axon/libaxon_pjrt.so")
        if hook is not None:
            mod.set_axon_ntff_profile_hook(hook)
    except Exception:
        pass


_ensure_ntff_hook()

import concourse.bass as bass
import concourse.tile as tile
from concourse import bacc, mybir
from concourse.bass_utils import run_bass_kernel_spmd

# Problem dims (hardcoded per the harness contract)
B, S, D, H, O, E, K = 4, 1024, 1024, 2048, 1024, 8, 2
T = B * S
LOAD_BALANCE_WEIGHT = 0.01
P = 128
KD = D // P    # 8  k-chunks for layer 1
KH = H // P    # 16 k-chunks for layers 2/3
MH = H // P    # 16 m-blocks for layers 1/2
MO = O // P    # 8  m-blocks for layer 3
NCHUNK = 512   # PSUM bank = 512 fp32

F16 = mybir.dt.float16
F32 = mybir.dt.float32

_nc_cache: dict = {}
LAST_RESULT = None


def _chunks(c):
    return [(o, min(NCHUNK, c - o)) for o in range(0, c, NCHUNK)]


def build_nc(C: int):
    """One expert's MLP over C (padded) tokens: yT = MLP(xT) in m-blocks."""
    nc = bacc.Bacc("TRN2", target_bir_lowering=False, debug=False, num_devices=E)

    xp = nc.declare_dram_parameter("xp", [P, KD, C], F16, isOutput=False)
    w1p = nc.declare_dram_parameter("w1p", [MH, P, KD, P], F16, isOutput=False)
    w2p = nc.declare_dram_parameter("w2p", [MH, P, KH, P], F16, isOutput=False)
    w3p = nc.declare_dram_parameter("w3p", [MO, P, KH, P], F16, isOutput=False)
    b1p = nc.declare_dram_parameter("b1p", [P, MH], F32, isOutput=False)
    b2p = nc.declare_dram_parameter("b2p", [P, MH], F32, isOutput=False)
    b3p = nc.declare_dram_parameter("b3p", [P, MO], F32, isOutput=False)
    ytp = nc.declare_dram_parameter("ytp", [MO, P, C], F32, isOutput=True)

    nch = _chunks(C)

    with tile.TileContext(nc) as tc, ExitStack() as ctx:
        consts = ctx.enter_context(tc.tile_pool(name="consts", bufs=1))
        acts = ctx.enter_context(tc.tile_pool(name="acts", bufs=1))
        wpool = ctx.enter_context(tc.tile_pool(name="wpool", bufs=6))
        ypool = ctx.enter_context(tc.tile_pool(name="ypool", bufs=4))
        pspool = ctx.enter_context(tc.tile_pool(name="pspool", bufs=8, space="PSUM"))

        x_sb = acts.tile([P, KD, C], F16, tag="x")
        # split the load per k-chunk so layer 1 can start after the first
        # slice; sync engine = HWDGE (fast first-byte), parallel to weight
        # loads on scalar's HWDGE ring
        for k in range(KD):
            nc.sync.dma_start(out=x_sb[:, k, :], in_=xp[:, k, :])

        b1_sb = consts.tile([P, MH], F32, tag="b1")
        b2_sb = consts.tile([P, MH], F32, tag="b2")
        b3_sb = consts.tile([P, MO], F32, tag="b3")
        nc.gpsimd.dma_start(out=b1_sb, in_=b1p[:, :])
        nc.gpsimd.dma_start(out=b2_sb, in_=b2p[:, :])
        nc.gpsimd.dma_start(out=b3_sb, in_=b3p[:, :])

        h1_sb = acts.tile([P, MH, C], F16, tag="h1")
        h2_sb = acts.tile([P, MH, C], F16, tag="h2")

        def layer(w_param, nk, nm, rhs_sb, bias_sb, wtag, evict):
            for m in range(nm):
                w_sb = wpool.tile([P, nk, P], F16, tag=wtag)
                nc.scalar.dma_start(out=w_sb, in_=w_param[m])
                psums = []
                for n, (off, nsz) in enumerate(nch):
                    ps_t = pspool.tile([P, NCHUNK], F32, tag="ps", name=f"ps_{m}_{n}")
                    psums.append(ps_t)
                for k in range(nk):
                    for n, (off, nsz) in enumerate(nch):
                        nc.tensor.matmul(
                            psums[n][:, :nsz],
                            w_sb[:, k, :],
                            rhs_sb[:, k, off:off + nsz],
                            start=(k == 0),
                            stop=(k == nk - 1),
                        )
                for n, (off, nsz) in enumerate(nch):
                    evict(m, off, nsz, psums[n])

        def layer1_nouter(w_param, rhs_sb, evict):
            # n-outer: the first matmul only needs x k-slice 0, and each
            # x k-slice is consumed over a full 16-m sweep (~3.4us), so x
            # DMA never stalls the PE. w1 stays resident (loaded once).
            # sync queue: shared with x (2.2MB, done ~12us) but NOT with the
            # eager w2 prefetches on scalar's queue, which would starve the
            # early w1 m-blocks the L1 m-sweep needs just-in-time
            w_sb = acts.tile([P, KD, H], F16, tag="w1r", name="w1_resident")
            for m in range(MH):
                nc.scalar.dma_start(out=w_sb[:, :, m * P:(m + 1) * P],
                                    in_=w_param[m])
            for n, (off, nsz) in enumerate(nch):
                for m in range(MH):
                    ps_t = pspool.tile([P, NCHUNK], F32, tag="ps",
                                       name=f"l1ps_{m}_{n}")
                    for k in range(KD):
                        nc.tensor.matmul(
                            ps_t[:, :nsz],
                            w_sb[:, k, m * P:(m + 1) * P],
                            rhs_sb[:, k, off:off + nsz],
                            start=(k == 0),
                            stop=(k == KD - 1),
                        )
                    evict(m, off, nsz, ps_t)

        def evict_relu(h_sb, bias_sb):
            def f(m, off, nsz, ps):
                nc.scalar.activation(
                    h_sb[:, m, off:off + nsz], ps[:, :nsz],
                    mybir.ActivationFunctionType.Relu,
                    bias=bias_sb[:, m:m + 1],
                )
            return f

        def evict_out(m, off, nsz, ps):
            y_sb = ypool.tile([P, NCHUNK], F32, tag="y")
            nc.scalar.activation(
                y_sb[:, :nsz], ps[:, :nsz],
                mybir.ActivationFunctionType.Identity,
                bias=b3_sb[:, m:m + 1],
            )
            nc.gpsimd.dma_start(out=ytp[m][:, off:off + nsz], in_=y_sb[:, :nsz])

        layer1_nouter(w1p, x_sb, evict_relu(h1_sb, b1_sb))
        layer(w2p, KH, MH, h1_sb, b2_sb, "w2", evict_relu(h2_sb, b2_sb))
        layer(w3p, KH, MO, h2_sb, b3_sb, "w3", evict_out)

    nc.compile()
    return nc


def _get_nc(C: int):
    if C not in _nc_cache:
        _nc_cache[C] = build_nc(C)
    return _nc_cache[C]


def _pack_w(w, nk, nm):
    """[nk*P, nm*P] fp32 -> [nm, P, nk, P] bf16 (m-block-major, partition-major)."""
    wb = w.astype(np.float16)
    return np.ascontiguousarray(
        wb.reshape(nk, P, nm, P).transpose(2, 1, 0, 3)
    )


def _pack_b(b, nm):
    """[nm*P] fp32 -> [P, nm] fp32."""
    return np.ascontiguousarray(b.astype(np.float32).reshape(nm, P).T)


def _softmax(x, axis=-1):
    m = np.max(x, axis=axis, keepdims=True)
    e = np.exp(x - m)
    return e / np.sum(e, axis=axis, keepdims=True)


def kernel(x, Wg, bg, W1, b1, W2, b2, W3, b3):
    x = np.asarray(x, dtype=np.float32)
    Wg = np.asarray(Wg, dtype=np.float32)
    bg = np.asarray(bg, dtype=np.float32)
    W1 = np.asarray(W1, dtype=np.float32)
    b1 = np.asarray(b1, dtype=np.float32)
    W2 = np.asarray(W2, dtype=np.float32)
    b2 = np.asarray(b2, dtype=np.float32)
    W3 = np.asarray(W3, dtype=np.float32)
    b3 = np.asarray(b3, dtype=np.float32)

    xt = x.reshape(T, D)

    # ---- gating (host, exact fp32) ----
    raw_gates = xt @ Wg + bg                                   # [T, E]
    topk_idx = np.argpartition(raw_gates, E - K, axis=-1)[:, E - K:]   # [T, K]
    topk_val = np.take_along_axis(raw_gates, topk_idx, axis=-1)
    gates = _softmax(topk_val, axis=-1)                        # [T, K]

    combine = np.zeros((T, E), dtype=np.float32)
    np.put_along_axis(combine, topk_idx, gates, axis=-1)

    # ---- aux outputs (gating-only) ----
    gate_probs = _softmax(raw_gates, axis=-1)                  # [T, E]
    avg_gate_probs = gate_probs.mean(axis=0, dtype=np.float32)
    avg_expert_counts = combine.mean(axis=0, dtype=np.float32)
    load_balance_loss = np.float32(
        LOAD_BALANCE_WEIGHT * E * np.sum(avg_gate_probs * avg_expert_counts,
                                         dtype=np.float32))
    gate_entropy = np.float32(
        -np.sum(avg_gate_probs * np.log(avg_gate_probs + 1e-8),
                dtype=np.float32))

    # ---- dispatch: token lists per expert ----
    tok_idx = [np.nonzero(combine[:, e] > 0)[0] for e in range(E)]
    counts = [len(ti) for ti in tok_idx]
    # capacity = exact max count rounded to 16 (remainder matmul chunks run
    # at full rate, so padding cost is linear in C)
    C = max(128, ((max(counts) + 15) // 16) * 16)

    xb = xt.astype(np.float16)
    in_maps = []
    for e in range(E):
        ti = tok_idx[e]
        xe = np.zeros((P, KD, C), dtype=np.float16)
        # gathered tokens, feature-major: [D, cnt] -> [P, KD, cnt]
        xe_g = xb[ti].T.reshape(KD, P, len(ti)).transpose(1, 0, 2)
        xe[:, :, :len(ti)] = xe_g
        in_maps.append({
            "xp": xe,
            "w1p": _pack_w(W1[e], KD, MH),
            "w2p": _pack_w(W2[e], KH, MH),
            "w3p": _pack_w(W3[e], KH, MO),
            "b1p": _pack_b(b1[e], MH),
            "b2p": _pack_b(b2[e], MH),
            "b3p": _pack_b(b3[e], MO),
        })

    # ---- run expert MLPs on the 8 cores ----
    nc = _get_nc(C)
    res = run_bass_kernel_spmd(nc, in_maps, list(range(E)))
    global LAST_RESULT
    LAST_RESULT = res

    # ---- combine with gates (host) ----
    out = np.zeros((T, O), dtype=np.float32)
    for e in range(E):
        ti = tok_idx[e]
        yt = res.results[e]["ytp"].reshape(O, C)   # [O, C]
        w_e = combine[ti, e]
        out[ti] += w_e[:, None] * yt[:, :len(ti)].T

    return (out.reshape(B, S, O), load_balance_loss, avg_expert_counts,
            gate_entropy)


# revision 19
# speedup vs baseline: 1.0906x; 1.0054x over previous
"""MoE kernel for trn2: expert-parallel over 8 NeuronCores.

Reference model (B,S,D,H,O,E,K = 4,1024,1024,2048,1024,8,2):
  gating: raw = x@Wg+bg; top-2 softmax -> combine weights
  experts: y_e = relu(relu(x@W1e+b1e)@W2e+b2e)@W3e+b3e
  out[t] = sum_e combine[t,e] * y_e[t]
  + aux outputs (load-balance loss, avg expert counts, gate entropy)
  (all derived from gating only)

Strategy: gating + dispatch on host (0.4% of FLOPs), one expert per core.
Each core computes its expert's MLP over only the tokens routed to it
(~T*K/E = 1024 of 4096 tokens -> 4x sparser than the reference's dense
grouped GEMM). Device matmuls in bf16 with fp32 PSUM accumulation.
"""

import os
import sys
import types

sys.path.insert(0, "/opt/trn_rl_repo")

import numpy as np
import ml_dtypes
from contextlib import ExitStack


def _ensure_ntff_hook():
    """Make trace=True work under axon if the image lacks antenv.axon_hooks."""
    try:
        from antenv.axon_hooks import get_axon_ntff_profile_hook  # noqa: F401
        return
    except ImportError:
        pass
    mod = types.ModuleType("antenv.axon_hooks")
    _hook = [None]
    mod.set_axon_ntff_profile_hook = lambda h: _hook.__setitem__(0, h)
    mod.get_axon_ntff_profile_hook = lambda: _hook[0]
    try:
        import antenv
        sys.modules["antenv.axon_hooks"] = mod
        antenv.axon_hooks = mod
        from trn_agent_boot.trn_boot import _ntff_profile_via_ctypes
        hook = _ntff_profile_via_ctypes("/opt/axon/libaxon_pjrt.so")
        if hook is not None:
            mod.set_axon_ntff_profile_hook(hook)
    except Exception:
        pass


_ensure_ntff_hook()

import concourse.bass as bass
import concourse.tile as tile
from concourse import bacc, mybir
from concourse.bass_utils import run_bass_kernel_spmd

# Problem dims (hardcoded per the harness contract)
B, S, D, H, O, E, K = 4, 1024, 1024, 2048, 1024, 8, 2
T = B * S
LOAD_BALANCE_WEIGHT = 0.01
P = 128
KD = D // P    # 8  k-chunks for layer 1
KH = H // P    # 16 k-chunks for layers 2/3
MH = H // P    # 16 m-blocks for layers 1/2
MO = O // P    # 8  m-blocks for layer 3
NCHUNK = 512   # PSUM bank = 512 fp32

F16 = mybir.dt.float16
F32 = mybir.dt.float32

_nc_cache: dict = {}
LAST_RESULT = None


def _chunks(c):
    return [(o, min(NCHUNK, c - o)) for o in range(0, c, NCHUNK)]


def build_nc(C: int):
    """One expert's MLP over C (padded) tokens: yT = MLP(xT) in m-blocks."""
    nc = bacc.Bacc("TRN2", target_bir_lowering=False, debug=False, num_devices=E)

    xp = nc.declare_dram_parameter("xp", [P, KD, C], F16, isOutput=False)
    w1p = nc.declare_dram_parameter("w1p", [MH, P, KD, P], F16, isOutput=False)
    w2p = nc.declare_dram_parameter("w2p", [MH, P, KH, P], F16, isOutput=False)
    w3p = nc.declare_dram_parameter("w3p", [MO, P, KH, P], F16, isOutput=False)
    b1p = nc.declare_dram_parameter("b1p", [P, MH], F32, isOutput=False)
    b2p = nc.declare_dram_parameter("b2p", [P, MH], F32, isOutput=False)
    b3p = nc.declare_dram_parameter("b3p", [P, MO], F32, isOutput=False)
    ytp = nc.declare_dram_parameter("ytp", [MO, P, C], F32, isOutput=True)

    nch = _chunks(C)

    with tile.TileContext(nc) as tc, ExitStack() as ctx:
        consts = ctx.enter_context(tc.tile_pool(name="consts", bufs=1))
        acts = ctx.enter_context(tc.tile_pool(name="acts", bufs=1))
        wpool = ctx.enter_context(tc.tile_pool(name="wpool", bufs=8))
        ypool = ctx.enter_context(tc.tile_pool(name="ypool", bufs=4))
        pspool = ctx.enter_context(tc.tile_pool(name="pspool", bufs=8, space="PSUM"))

        x_sb = acts.tile([P, KD, C], F16, tag="x")
        # split the load per k-chunk so layer 1 can start after the first
        # slice; sync engine = HWDGE (fast first-byte), parallel to weight
        # loads on scalar's HWDGE ring
        for k in range(KD):
            nc.sync.dma_start(out=x_sb[:, k, :], in_=xp[:, k, :])

        b1_sb = consts.tile([P, MH], F32, tag="b1")
        b2_sb = consts.tile([P, MH], F32, tag="b2")
        b3_sb = consts.tile([P, MO], F32, tag="b3")
        nc.gpsimd.dma_start(out=b1_sb, in_=b1p[:, :])
        nc.gpsimd.dma_start(out=b2_sb, in_=b2p[:, :])
        nc.gpsimd.dma_start(out=b3_sb, in_=b3p[:, :])

        h1_sb = acts.tile([P, MH, C], F16, tag="h1")
        h2_sb = acts.tile([P, MH, C], F16, tag="h2")

        def layer(w_param, nk, nm, rhs_sb, bias_sb, wtag, evict):
            for m in range(nm):
                w_sb = wpool.tile([P, nk, P], F16, tag=wtag)
                nc.scalar.dma_start(out=w_sb, in_=w_param[m])
                psums = []
                for n, (off, nsz) in enumerate(nch):
                    ps_t = pspool.tile([P, NCHUNK], F32, tag="ps", name=f"ps_{m}_{n}")
                    psums.append(ps_t)
                for k in range(nk):
                    for n, (off, nsz) in enumerate(nch):
                        nc.tensor.matmul(
                            psums[n][:, :nsz],
                            w_sb[:, k, :],
                            rhs_sb[:, k, off:off + nsz],
                            start=(k == 0),
                            stop=(k == nk - 1),
                        )
                for n, (off, nsz) in enumerate(nch):
                    evict(m, off, nsz, psums[n])

        def layer1_nouter(w_param, rhs_sb, evict):
            # n-outer: the first matmul only needs x k-slice 0, and each
            # x k-slice is consumed over a full 16-m sweep (~3.4us), so x
            # DMA never stalls the PE. w1 stays resident (loaded once).
            # sync queue: shared with x (2.2MB, done ~12us) but NOT with the
            # eager w2 prefetches on scalar's queue, which would starve the
            # early w1 m-blocks the L1 m-sweep needs just-in-time
            w_sb = acts.tile([P, KD, H], F16, tag="w1r", name="w1_resident")
            for m in range(MH):
                nc.scalar.dma_start(out=w_sb[:, :, m * P:(m + 1) * P],
                                    in_=w_param[m])
            for n, (off, nsz) in enumerate(nch):
                for m in range(MH):
                    ps_t = pspool.tile([P, NCHUNK], F32, tag="ps",
                                       name=f"l1ps_{m}_{n}")
                    for k in range(KD):
                        nc.tensor.matmul(
                            ps_t[:, :nsz],
                            w_sb[:, k, m * P:(m + 1) * P],
                            rhs_sb[:, k, off:off + nsz],
                            start=(k == 0),
                            stop=(k == KD - 1),
                        )
                    evict(m, off, nsz, ps_t)

        def evict_relu(h_sb, bias_sb):
            def f(m, off, nsz, ps):
                nc.scalar.activation(
                    h_sb[:, m, off:off + nsz], ps[:, :nsz],
                    mybir.ActivationFunctionType.Relu,
                    bias=bias_sb[:, m:m + 1],
                )
            return f

        def evict_out(m, off, nsz, ps):
            y_sb = ypool.tile([P, NCHUNK], F32, tag="y")
            nc.scalar.activation(
                y_sb[:, :nsz], ps[:, :nsz],
                mybir.ActivationFunctionType.Identity,
                bias=b3_sb[:, m:m + 1],
            )
            nc.gpsimd.dma_start(out=ytp[m][:, off:off + nsz], in_=y_sb[:, :nsz])

        layer1_nouter(w1p, x_sb, evict_relu(h1_sb, b1_sb))
        layer(w2p, KH, MH, h1_sb, b2_sb, "w2", evict_relu(h2_sb, b2_sb))
        layer(w3p, KH, MO, h2_sb, b3_sb, "w3", evict_out)

    nc.compile()
    return nc


def _get_nc(C: int):
    if C not in _nc_cache:
        _nc_cache[C] = build_nc(C)
    return _nc_cache[C]


def _pack_w(w, nk, nm):
    """[nk*P, nm*P] fp32 -> [nm, P, nk, P] bf16 (m-block-major, partition-major)."""
    wb = w.astype(np.float16)
    return np.ascontiguousarray(
        wb.reshape(nk, P, nm, P).transpose(2, 1, 0, 3)
    )


def _pack_b(b, nm):
    """[nm*P] fp32 -> [P, nm] fp32."""
    return np.ascontiguousarray(b.astype(np.float32).reshape(nm, P).T)


def _softmax(x, axis=-1):
    m = np.max(x, axis=axis, keepdims=True)
    e = np.exp(x - m)
    return e / np.sum(e, axis=axis, keepdims=True)


def kernel(x, Wg, bg, W1, b1, W2, b2, W3, b3):
    x = np.asarray(x, dtype=np.float32)
    Wg = np.asarray(Wg, dtype=np.float32)
    bg = np.asarray(bg, dtype=np.float32)
    W1 = np.asarray(W1, dtype=np.float32)
    b1 = np.asarray(b1, dtype=np.float32)
    W2 = np.asarray(W2, dtype=np.float32)
    b2 = np.asarray(b2, dtype=np.float32)
    W3 = np.asarray(W3, dtype=np.float32)
    b3 = np.asarray(b3, dtype=np.float32)

    xt = x.reshape(T, D)

    # ---- gating (host, exact fp32) ----
    raw_gates = xt @ Wg + bg                                   # [T, E]
    topk_idx = np.argpartition(raw_gates, E - K, axis=-1)[:, E - K:]   # [T, K]
    topk_val = np.take_along_axis(raw_gates, topk_idx, axis=-1)
    gates = _softmax(topk_val, axis=-1)                        # [T, K]

    combine = np.zeros((T, E), dtype=np.float32)
    np.put_along_axis(combine, topk_idx, gates, axis=-1)

    # ---- aux outputs (gating-only) ----
    gate_probs = _softmax(raw_gates, axis=-1)                  # [T, E]
    avg_gate_probs = gate_probs.mean(axis=0, dtype=np.float32)
    avg_expert_counts = combine.mean(axis=0, dtype=np.float32)
    load_balance_loss = np.float32(
        LOAD_BALANCE_WEIGHT * E * np.sum(avg_gate_probs * avg_expert_counts,
                                         dtype=np.float32))
    gate_entropy = np.float32(
        -np.sum(avg_gate_probs * np.log(avg_gate_probs + 1e-8),
                dtype=np.float32))

    # ---- dispatch: token lists per expert ----
    tok_idx = [np.nonzero(combine[:, e] > 0)[0] for e in range(E)]
    counts = [len(ti) for ti in tok_idx]
    # capacity = exact max count rounded to 16 (remainder matmul chunks run
    # at full rate, so padding cost is linear in C)
    C = max(128, ((max(counts) + 15) // 16) * 16)

    xb = xt.astype(np.float16)
    in_maps = []
    for e in range(E):
        ti = tok_idx[e]
        xe = np.zeros((P, KD, C), dtype=np.float16)
        # gathered tokens, feature-major: [D, cnt] -> [P, KD, cnt]
        xe_g = xb[ti].T.reshape(KD, P, len(ti)).transpose(1, 0, 2)
        xe[:, :, :len(ti)] = xe_g
        in_maps.append({
            "xp": xe,
            "w1p": _pack_w(W1[e], KD, MH),
            "w2p": _pack_w(W2[e], KH, MH),
            "w3p": _pack_w(W3[e], KH, MO),
            "b1p": _pack_b(b1[e], MH),
            "b2p": _pack_b(b2[e], MH),
            "b3p": _pack_b(b3[e], MO),
        })

    # ---- run expert MLPs on the 8 cores ----
    nc = _get_nc(C)
    res = run_bass_kernel_spmd(nc, in_maps, list(range(E)))
    global LAST_RESULT
    LAST_RESULT = res

    # ---- combine with gates (host) ----
    out = np.zeros((T, O), dtype=np.float32)
    for e in range(E):
        ti = tok_idx[e]
        yt = res.results[e]["ytp"].reshape(O, C)   # [O, C]
        w_e = combine[ti, e]
        out[ti] += w_e[:, None] * yt[:, :len(ti)].T

    return (out.reshape(B, S, O), load_balance_loss, avg_expert_counts,
            gate_entropy)
